# revision 5
# baseline (speedup 1.0000x reference)
"""Trainium2 Bass kernel for nn_AttentionMarketEncoder.

Takes FULL unsharded inputs, shards windows across 8 NeuronCores
(24 windows/core incl. padding; W padded 180->192), runs a Bass/Tile
kernel per core, and combines partial window-reductions on the host.

Self-contained: hardcodes all shapes; no sibling imports.
"""
import math
import sys

sys.path.insert(0, "/opt/trn_rl_repo")

import numpy as np

import concourse.bacc as bacc
import concourse.tile as tile
from concourse import mybir
from concourse.bass_utils import run_bass_kernel_spmd

F32 = mybir.dt.float32
F32R = mybir.dt.float32r
BF16 = mybir.dt.bfloat16

W = 180
B = 256
D = 256
H = 8
DK = 32
L = 4
N_CORES = 8
W_LOC = 24            # windows per core (incl. pad)
N_PAIR = W_LOC // 2   # window pairs per core
EPS = 1e-9

_CACHE = {}


def _build(n_pair):
    nc = bacc.Bacc("TRN2", target_bir_lowering=False, debug=False,
                   num_devices=N_CORES)

    # ---- DRAM inputs (per core) ----
    mv = nc.dram_tensor("mv", [n_pair, 8, 512], F32, kind="ExternalInput").ap()
    fcbar = nc.dram_tensor("fcbar", [8, 256], F32, kind="ExternalInput").ap()
    wq_d = nc.dram_tensor("wq", [128, 2, 256], F32, kind="ExternalInput").ap()
    wk_d = nc.dram_tensor("wk", [128, 2, 256], F32, kind="ExternalInput").ap()
    wv_d = nc.dram_tensor("wv", [128, 2, 256], F32, kind="ExternalInput").ap()
    wo_d = nc.dram_tensor("wo", [128, 2, 256], F32, kind="ExternalInput").ap()
    fc1_d = nc.dram_tensor("fc1", [128, L, 2, 256], F32, kind="ExternalInput").ap()
    fc2_d = nc.dram_tensor("fc2", [128, L, 2, 256], F32, kind="ExternalInput").ap()
    b1_d = nc.dram_tensor("b1", [1, L, 256], F32, kind="ExternalInput").ap()
    b2_d = nc.dram_tensor("b2", [1, L, 256], F32, kind="ExternalInput").ap()
    g2_d = nc.dram_tensor("g2", [1, L, 256], F32, kind="ExternalInput").ap()
    b2c_d = nc.dram_tensor("b2c", [128, L, 2], F32, kind="ExternalInput").ap()
    ident_d = nc.dram_tensor("ident", [128, 128], F32, kind="ExternalInput").ap()
    fcf_d = nc.dram_tensor("fcf", [128, W_LOC], F32, kind="ExternalInput").ap()
    acc_d = nc.dram_tensor("acc", [128, 2, 256], F32, kind="ExternalOutput").ap()

    EXP = mybir.ActivationFunctionType.Exp
    LN = mybir.ActivationFunctionType.Ln
    MULT = mybir.AluOpType.mult
    ADD = mybir.AluOpType.add
    MAX = mybir.AluOpType.max

    with tile.TileContext(nc) as tc:
        with tc.tile_pool(name="consts", bufs=1) as consts, \
             tc.tile_pool(name="sbA", bufs=2) as sbA, \
             tc.tile_pool(name="sbB", bufs=2) as sbB, \
             tc.tile_pool(name="sbC", bufs=1) as sbC, \
             tc.tile_pool(name="psA", bufs=2, space="PSUM") as psA, \
             tc.tile_pool(name="psB", bufs=2, space="PSUM") as psB, \
             tc.tile_pool(name="psC", bufs=2, space="PSUM") as psC:

            # ---- constants / weights in SBUF ----
            fcbar_sb = consts.tile([8, 256], F32R, tag="fcbar")
            nc.sync.dma_start(out=fcbar_sb, in_=fcbar.bitcast(F32R))
            wq = consts.tile([128, 2, 256], F32R, tag="wq")
            wk = consts.tile([128, 2, 256], F32R, tag="wk")
            wv = consts.tile([128, 2, 256], F32R, tag="wv")
            wo = consts.tile([128, 2, 256], F32R, tag="wo")
            nc.sync.dma_start(out=wq, in_=wq_d.bitcast(F32R))
            nc.sync.dma_start(out=wk, in_=wk_d.bitcast(F32R))
            nc.sync.dma_start(out=wv, in_=wv_d.bitcast(F32R))
            nc.sync.dma_start(out=wo, in_=wo_d.bitcast(F32R))
            fc1 = consts.tile([128, L, 2, 256], F32R, tag="fc1")
            fc2 = consts.tile([128, L, 2, 256], F32R, tag="fc2")
            nc.sync.dma_start(out=fc1, in_=fc1_d.bitcast(F32R))
            nc.sync.dma_start(out=fc2, in_=fc2_d.bitcast(F32R))
            b1r = consts.tile([1, L, 256], F32R, tag="b1r")
            b2r = consts.tile([1, L, 256], F32R, tag="b2r")
            g2r = consts.tile([1, L, 256], F32R, tag="g2r")
            nc.sync.dma_start(out=b1r, in_=b1_d.bitcast(F32R))
            nc.sync.dma_start(out=b2r, in_=b2_d.bitcast(F32R))
            nc.sync.dma_start(out=g2r, in_=g2_d.bitcast(F32R))
            b2c = consts.tile([128, L, 2], F32, tag="b2c")
            nc.sync.dma_start(out=b2c, in_=b2c_d)
            ident = consts.tile([128, 128], F32R, tag="ident")
            nc.sync.dma_start(out=ident, in_=ident_d.bitcast(F32R))
            fcf = consts.tile([128, W_LOC], F32, tag="fcf")
            nc.sync.dma_start(out=fcf, in_=fcf_d)

            ones32 = consts.tile([128, 32], BF16, tag="ones32")
            nc.vector.memset(ones32, 1.0)
            ones128 = consts.tile([128, 1], F32R, tag="ones128")
            nc.vector.memset(ones128.bitcast(F32), 1.0)
            ones1 = consts.tile([1, 128], F32R, tag="ones1")
            nc.vector.memset(ones1.bitcast(F32), 1.0)
            onesrow = consts.tile([1, 512], F32R, tag="onesrow")
            nc.vector.memset(onesrow.bitcast(F32), 1.0)

            acc = [consts.tile([128, 2, 256], F32, tag="acc0", name="acc0"),
                   consts.tile([128, 2, 256], F32, tag="acc1", name="acc1")]
            nc.gpsimd.memset(acc[0], 0.0)

            def norm_stats(t_sb, sq_sb, tag):
                """t_sb [128,2,512] f32r -> (r_row, mrn_row) [1,512] f32r."""
                for dt in range(2):
                    nc.gpsimd.tensor_mul(sq_sb[:, dt, :],
                                         t_sb[:, dt, :].bitcast(F32),
                                         t_sb[:, dt, :].bitcast(F32))
                u_st = psC.tile([1, 512], F32, tag="misc", name=f"u{tag}")
                q_st = psC.tile([1, 512], F32, tag="misc", name=f"q{tag}")
                for k in range(2):
                    nc.tensor.matmul(u_st, ones128, t_sb[:, k, :],
                                     start=(k == 0), stop=(k == 1))
                for k in range(2):
                    nc.tensor.matmul(q_st, ones128, sq_sb[:, k, :],
                                     start=(k == 0), stop=(k == 1))
                m_row = sbB.tile([1, 512], F32R, tag="m_row", name=f"m{tag}")
                nc.vector.tensor_scalar(out=m_row, in0=u_st,
                                        scalar1=1.0 / 256.0, scalar2=None,
                                        op0=MULT)
                m2_row = sbB.tile([1, 512], F32, tag="m2_row", name=f"m2{tag}")
                nc.gpsimd.tensor_mul(m2_row, m_row.bitcast(F32),
                                     m_row.bitcast(F32))
                vraw = sbB.tile([1, 512], F32, tag="vraw", name=f"v{tag}")
                nc.vector.scalar_tensor_tensor(out=vraw, in0=m2_row,
                                               scalar=-256.0, op0=MULT,
                                               in1=q_st, op1=ADD)
                lnv = sbB.tile([1, 512], F32, tag="lnv", name=f"l{tag}")
                nc.scalar.activation(out=lnv, in_=vraw, func=LN,
                                     scale=1.0 / 255.0)
                r_row = sbB.tile([1, 512], F32R, tag="r_row", name=f"r{tag}")
                nc.scalar.activation(out=r_row, in_=lnv, func=EXP, scale=-0.5)
                mrn_row = sbB.tile([1, 512], F32R, tag="mrn_row", name=f"n{tag}")
                nc.vector.scalar_tensor_tensor(out=mrn_row,
                                               in0=m_row.bitcast(F32),
                                               scalar=-1.0, op0=MULT,
                                               in1=r_row.bitcast(F32), op1=MULT)
                return r_row, mrn_row

            # ================= main loop over window pairs =================
            for g in range(n_pair):
                # ---- x projection ----
                mvT = sbB.tile([8, 512], F32R, tag="mvT")
                nc.sync.dma_start(out=mvT, in_=mv[g].bitcast(F32R))
                ps_x = psA.tile([128, 1024], F32, tag="big", name=f"x{g}")
                for dt in range(2):
                    nc.tensor.matmul(ps_x[:, 512 * dt:512 * dt + 512],
                                     fcbar_sb[:, 128 * dt:128 * dt + 128],
                                     mvT, start=True, stop=True)
                xt = sbA.tile([128, 2, 512], F32R, tag="xt", name=f"xt{g}")
                for dt in range(2):
                    nc.vector.tensor_copy(xt[:, dt, :],
                                          ps_x[:, 512 * dt:512 * dt + 512])

                for j in range(L):
                    tg = f"{g}_{j}"
                    # ---- Q/K/V projections ----
                    ps_q = psA.tile([128, 1024], F32, tag="big", name=f"q{tg}")
                    ps_k = psA.tile([128, 1024], F32, tag="big", name=f"k{tg}")
                    ps_v = psA.tile([128, 1024], F32, tag="big", name=f"v{tg}")
                    for m in range(2):
                        for k in range(2):
                            nc.tensor.matmul(ps_q[:, 512 * m:512 * m + 512],
                                             wq[:, k, 128 * m:128 * m + 128],
                                             xt[:, k, :],
                                             start=(k == 0), stop=(k == 1))
                            nc.tensor.matmul(ps_k[:, 512 * m:512 * m + 512],
                                             wk[:, k, 128 * m:128 * m + 128],
                                             xt[:, k, :],
                                             start=(k == 0), stop=(k == 1))
                    # V: out quarters (bt, w) at cols 256*(2*bt+w)
                    for bt in range(2):
                        for w in range(2):
                            for k in range(2):
                                c0 = 256 * (2 * bt + w)
                                nc.tensor.matmul(
                                    ps_v[:, c0:c0 + 256],
                                    xt[:, k, 256 * w + 128 * bt:
                                       256 * w + 128 * bt + 128],
                                    wv[:, k, :],
                                    start=(w == 0 and k == 0),
                                    stop=(w == 1 and k == 1))
                    qt = sbB.tile([128, 2, 512], BF16, tag="qt")
                    kt = sbB.tile([128, 2, 512], BF16, tag="kt")
                    for m in range(2):
                        nc.vector.tensor_copy(qt[:, m, :],
                                              ps_q[:, 512 * m:512 * m + 512])
                        nc.vector.tensor_copy(kt[:, m, :],
                                              ps_k[:, 512 * m:512 * m + 512])
                    v_sb = sbB.tile([128, 2, 2, 256], BF16, tag="v_sb")
                    for bt in range(2):
                        nc.vector.tensor_copy(v_sb[:, bt, :, :],
                                              ps_v[:, 512 * bt:512 * bt + 512])
                    # ---- partition remap to [32, 8, 512] ----
                    q32 = sbC.tile([32, 8, 512], BF16, tag="q32")
                    k32 = sbC.tile([32, 8, 512], BF16, tag="k32")
                    for m in range(2):
                        for hh in range(4):
                            h = 4 * m + hh
                            nc.sync.dma_start(
                                out=q32[:, h, :],
                                in_=qt[32 * hh:32 * hh + 32, m, :])
                            nc.sync.dma_start(
                                out=k32[:, h, :],
                                in_=kt[32 * hh:32 * hh + 32, m, :])
                    # ---- scores + exp ----
                    exps = sbC.tile([128, 2, 2, 8, 256], BF16, tag="exps")
                    for w in range(2):
                        for ct in range(2):
                            for hp in range(4):
                                ps_s = psB.tile([128, 512], F32, tag="score",
                                                name=f"s{tg}_{w}{ct}{hp}")
                                for hh in range(2):
                                    h = 2 * hp + hh
                                    nc.tensor.matmul(
                                        ps_s[:, 256 * hh:256 * hh + 256],
                                        k32[:, h, 256 * w + 128 * ct:
                                            256 * w + 128 * ct + 128],
                                        q32[:, h, 256 * w:256 * w + 256],
                                        start=(hh == 0), stop=(hh == 1))
                                nc.scalar.activation(
                                    out=exps[:, w, ct, 2 * hp:2 * hp + 2, :],
                                    in_=ps_s, func=EXP, scale=1.0)
                    # ---- sumexp + PV per (w, grp) ----
                    ht_sb = sbB.tile([128, 2, 2, 256], F32R, tag="ht_sb")
                    for w in range(2):
                        for grp in range(2):
                            ps_sh = psC.tile([128, 512], F32, tag="misc",
                                             name=f"sh{tg}_{w}{grp}")
                            for hq in range(4):
                                h = 4 * grp + hq
                                for ct in range(2):
                                    nc.tensor.matmul(
                                        ps_sh[32 * hq:32 * hq + 32, 0:256],
                                        ones32,
                                        exps[:, w, ct, h, :],
                                        start=(ct == 0), stop=False,
                                        tile_position=(0, 32 * hq))
                                for ct in range(2):
                                    nc.tensor.matmul(
                                        ps_sh[32 * hq:32 * hq + 32, 256:512],
                                        v_sb[:, ct, w, 32 * h:32 * h + 32],
                                        exps[:, w, ct, h, :],
                                        start=False, stop=(ct == 1),
                                        tile_position=(0, 32 * hq))
                            r_sb = sbB.tile([128, 256], F32, tag="r_sb",
                                            name=f"r{tg}_{w}{grp}")
                            nc.vector.reciprocal_approx_fast(
                                r_sb, ps_sh[:, 0:256])
                            nc.vector.tensor_mul(ht_sb[:, grp, w, :],
                                                 ps_sh[:, 256:512], r_sb)
                    # ---- WO + residual ----
                    ps_o = psA.tile([128, 1024], F32, tag="big", name=f"o{tg}")
                    for m in range(2):
                        for w in range(2):
                            for kc in range(2):
                                nc.tensor.matmul(
                                    ps_o[:, 512 * m + 256 * w:
                                         512 * m + 256 * w + 256],
                                    wo[:, kc, 128 * m:128 * m + 128],
                                    ht_sb[:, kc, w, :],
                                    start=(w == 0 and kc == 0), stop=False)
                        nc.tensor.matmul(ps_o[:, 512 * m:512 * m + 512],
                                         ident, xt[:, m, :],
                                         start=False, stop=True)
                    t_sb = sbB.tile([128, 2, 512], F32R, tag="t_sb")
                    for m in range(2):
                        nc.vector.tensor_copy(t_sb[:, m, :],
                                              ps_o[:, 512 * m:512 * m + 512])
                    # ---- norm1 (pure normalize; affine folded into fc1) ----
                    sq_sb = sbB.tile([128, 2, 512], F32R, tag="sq_sb")
                    r1, mrn1 = norm_stats(t_sb, sq_sb, f"n1{tg}")
                    mb1 = psA.tile([128, 1024], F32, tag="big", name=f"mb1{tg}")
                    nc.tensor.matmul(mb1[:, 0:512], ones1, r1,
                                     start=True, stop=True)
                    nc.tensor.matmul(mb1[:, 512:1024], ones1, mrn1,
                                     start=True, stop=True)
                    that = sbB.tile([128, 2, 512], F32R, tag="that")
                    for dt in range(2):
                        u_t = sbB.tile([128, 512], F32, tag="u_t",
                                       name=f"u1{tg}_{dt}")
                        nc.vector.tensor_mul(u_t, t_sb[:, dt, :].bitcast(F32),
                                             mb1[:, 0:512])
                        nc.vector.affine_then_add(out=that[:, dt, :],
                                                  in0=u_t,
                                                  in1=mb1[:, 512:1024],
                                                  scale=1.0, bias=0.0)
                    # ---- FFN1 + lrelu ----
                    ps_z = psA.tile([128, 1024], F32, tag="big", name=f"z{tg}")
                    for m in range(2):
                        for kc in range(2):
                            nc.tensor.matmul(ps_z[:, 512 * m:512 * m + 512],
                                             fc1[:, j, kc, 128 * m:128 * m + 128],
                                             that[:, kc, :],
                                             start=(kc == 0), stop=False)
                        nc.tensor.matmul(ps_z[:, 512 * m:512 * m + 512],
                                         b1r[0:1, j, 128 * m:128 * m + 128],
                                         onesrow, start=False, stop=True)
                    z_sb = sbB.tile([128, 2, 512], F32, tag="z_sb")
                    z01 = sbB.tile([128, 2, 512], F32, tag="z01")
                    h1 = sbB.tile([128, 2, 512], F32R, tag="h1")
                    for m in range(2):
                        nc.scalar.copy(z_sb[:, m, :],
                                       ps_z[:, 512 * m:512 * m + 512])
                        nc.gpsimd.tensor_scalar_mul(z01[:, m, :],
                                                    z_sb[:, m, :], 0.01)
                        nc.vector.tensor_max(h1[:, m, :], z_sb[:, m, :],
                                             z01[:, m, :])
                    # ---- FFN2 + bias + residual ----
                    ps_w = psA.tile([128, 1024], F32, tag="big", name=f"w{tg}")
                    for m in range(2):
                        for kc in range(2):
                            nc.tensor.matmul(ps_w[:, 512 * m:512 * m + 512],
                                             fc2[:, j, kc, 128 * m:128 * m + 128],
                                             h1[:, kc, :],
                                             start=(kc == 0), stop=False)
                        nc.tensor.matmul(ps_w[:, 512 * m:512 * m + 512],
                                         b2r[0:1, j, 128 * m:128 * m + 128],
                                         onesrow, start=False, stop=False)
                        nc.tensor.matmul(ps_w[:, 512 * m:512 * m + 512],
                                         ident, xt[:, m, :],
                                         start=False, stop=True)
                    vo_sb = sbB.tile([128, 2, 512], F32R, tag="vo_sb")
                    for m in range(2):
                        nc.vector.tensor_copy(vo_sb[:, m, :],
                                              ps_w[:, 512 * m:512 * m + 512])
                    # ---- norm2 with affine (g2 folded into bcast lhsT) ----
                    sq2_sb = sbB.tile([128, 2, 512], F32R, tag="sq_sb",
                                      name=f"sq2{tg}")
                    r2, mrn2 = norm_stats(vo_sb, sq2_sb, f"n2{tg}")
                    xt_next = sbA.tile([128, 2, 512], F32R, tag="xt",
                                       name=f"xt{g}_{j}")
                    for dt in range(2):
                        mb2 = psA.tile([128, 1024], F32, tag="big",
                                       name=f"mb2{tg}_{dt}")
                        nc.tensor.matmul(mb2[:, 0:512],
                                         g2r[0:1, j, 128 * dt:128 * dt + 128],
                                         r2, start=True, stop=True)
                        nc.tensor.matmul(mb2[:, 512:1024],
                                         g2r[0:1, j, 128 * dt:128 * dt + 128],
                                         mrn2, start=True, stop=True)
                        u2_t = sbB.tile([128, 512], F32, tag="u_t",
                                        name=f"u2{tg}_{dt}")
                        nc.vector.tensor_mul(u2_t,
                                             vo_sb[:, dt, :].bitcast(F32),
                                             mb2[:, 0:512])
                        nc.vector.affine_then_add(out=xt_next[:, dt, :],
                                                  in0=u2_t,
                                                  in1=mb2[:, 512:1024],
                                                  scale=1.0,
                                                  bias=b2c[:, j, dt:dt + 1])
                    xt = xt_next

                # ---- final reduction accumulate (ping-pong per window) ----
                for w in range(2):
                    wi = 2 * g + w
                    a_old, a_new = acc[wi % 2], acc[(wi + 1) % 2]
                    for dt in range(2):
                        nc.vector.scalar_tensor_tensor(
                            out=a_new[:, dt, :],
                            in0=xt[:, dt, 256 * w:256 * w + 256].bitcast(F32),
                            scalar=fcf[:, wi:wi + 1], op0=MULT,
                            in1=a_old[:, dt, :], op1=ADD)

            out_acc = acc[(2 * n_pair) % 2]
            nc.sync.dma_start(out=acc_d, in_=out_acc)

    nc.compile()
    return nc


def _prep_host(inputs):
    """Host-side weight preprocessing (fp32)."""
    mvals = np.asarray(inputs["market_values"], np.float32)      # [180,256,5]
    fcbar_W = np.asarray(inputs["fcbar_W"], np.float32)          # [6,256]
    fcbar_b = np.asarray(inputs["fcbar_b"], np.float32)
    WQ = np.asarray(inputs["WQ"], np.float32)                    # [8,256,32]
    WK = np.asarray(inputs["WK"], np.float32)
    WV = np.asarray(inputs["WV"], np.float32)
    WO = np.asarray(inputs["WO"], np.float32)                    # [256,256]
    a_gain = np.asarray(inputs["a_gain"], np.float32)            # [4,256]
    a_bias = np.asarray(inputs["a_bias"], np.float32)
    fc1_W = np.asarray(inputs["fc1_W"], np.float32)              # [4,256,256]
    fc1_b = np.asarray(inputs["fc1_b"], np.float32)
    fc2_W = np.asarray(inputs["fc2_W"], np.float32)
    fc2_b = np.asarray(inputs["fc2_b"], np.float32)
    fo_gain = np.asarray(inputs["fo_gain"], np.float32)
    fo_bias = np.asarray(inputs["fo_bias"], np.float32)
    fcf_W = np.asarray(inputs["fcf_W"], np.float32)              # [180,1]

    scale = 1.0 / math.sqrt(DK)

    def as_lhsT(w):  # [256, 256] -> [128, 2, 256]
        return np.ascontiguousarray(
            w.reshape(2, 128, 256).transpose(1, 0, 2))

    wq = as_lhsT((WQ * scale).transpose(1, 0, 2).reshape(256, 256))
    wk = as_lhsT(WK.transpose(1, 0, 2).reshape(256, 256))
    wv = as_lhsT(WV.transpose(1, 0, 2).reshape(256, 256))
    wo = as_lhsT(WO)

    fc1 = np.zeros((128, L, 2, 256), np.float32)
    fc2 = np.zeros((128, L, 2, 256), np.float32)
    b1 = np.zeros((1, L, 256), np.float32)
    b2 = np.zeros((1, L, 256), np.float32)
    g2 = np.zeros((1, L, 256), np.float32)
    b2c = np.zeros((128, L, 2), np.float32)
    for j in range(L):
        g1j = a_gain[j] + 1.0
        fc1p = fc1_W[j] * g1j[:, None]
        fc1[:, j] = as_lhsT(fc1p)
        b1[0, j] = fc1_b[j] + a_bias[j] @ fc1_W[j]
        fc2[:, j] = as_lhsT(fc2_W[j])
        b2[0, j] = fc2_b[j]
        g2[0, j] = fo_gain[j] + 1.0
        b2c[:, j, 0] = fo_bias[j][0:128]
        b2c[:, j, 1] = fo_bias[j][128:256]

    fcbar_aug = np.zeros((8, 256), np.float32)
    fcbar_aug[0:6] = fcbar_W
    fcbar_aug[6] = fcbar_b

    tags = ((np.arange(W, dtype=np.float32) - W / 2) / (W / 2))

    # per-core mv tensor [n_pair, 8, 512] and fcf [128, W_LOC]
    mv_cores, fcf_cores = [], []
    for c in range(N_CORES):
        mvc = np.zeros((N_PAIR, 8, 512), np.float32)
        fcfc = np.zeros((128, W_LOC), np.float32)
        for i in range(W_LOC):
            wg = W_LOC * c + i
            g, wslot = i // 2, i % 2
            sl = slice(256 * wslot, 256 * wslot + 256)
            if wg < W:
                mvc[g, 0:5, sl] = mvals[wg].T
                mvc[g, 5, sl] = tags[wg]
                fcfc[:, i] = fcf_W[wg, 0]
            else:
                mvc[g, 0:5, sl] = mvals[0].T
                mvc[g, 5, sl] = tags[0]
            mvc[g, 6, sl] = 1.0
        mv_cores.append(mvc)
        fcf_cores.append(fcfc)

    shared = {
        "fcbar": fcbar_aug, "wq": wq, "wk": wk, "wv": wv, "wo": wo,
        "fc1": fc1, "fc2": fc2, "b1": b1, "b2": b2, "g2": g2, "b2c": b2c,
        "ident": np.eye(128, dtype=np.float32),
    }
    in_maps = []
    for c in range(N_CORES):
        m = dict(shared)
        m["mv"] = mv_cores[c]
        m["fcf"] = fcf_cores[c]
        in_maps.append(m)
    fcf_b = np.asarray(inputs["fcf_b"], np.float32)
    return in_maps, fcf_b


def kernel(**inputs):
    in_maps, fcf_b = _prep_host(inputs)
    if "nc" not in _CACHE:
        _CACHE["nc"] = _build(N_PAIR)
    res = run_bass_kernel_spmd(_CACHE["nc"], in_maps,
                               core_ids=list(range(N_CORES)))
    total = np.zeros((128, 2, 256), np.float64)
    for c in range(N_CORES):
        total += res.results[c]["acc"].astype(np.float64)
    # acc[p, dt, b] -> out[dt*128+p, b]
    out = total.transpose(1, 0, 2).reshape(256, 256) + float(fcf_b[0])
    out = np.where(out > 0, out, 0.01 * out)
    return out.astype(np.float32)
